# revision 32
# baseline (speedup 1.0000x reference)
"""SSD Detect (decode + per-class top-200) Trainium2 Bass kernel.

Sharding: data-parallel over batch. 8 batches -> 8 NeuronCores, one batch per
core. Each core computes, for its batch:
  decoded boxes [25575, 4]  (SSD decode from loc + priors)
  per class c in [0, 81): top-200 scores (desc, ties -> lower prior index
  first, matching jax.lax.top_k) with their decoded boxes ->
  out[c, r] = [score_r, x1, y1, x2, y2]

Device algorithm per core:
  - conf is loaded as TWO half-window tiles (priors [200p, 200p+100) and
    [200p+100, 200p+200) per window), each half split across BOTH HWDGE
    queues so it uses all 16 DMA engines (~190GB/s). A DVE gate copy makes
    the h1 DMAs WAW-wait for h0, so h0 lands at ~30us with full bandwidth
    and L1 h0 overlaps the h1 load. Descriptor-heavy transfers (loc/priors:
    127 x 3.2KB; dma_start is a blocking DMA_DIRECT2D on its issuing
    engine) are queued after conf. <=64-partition slices keep 32.4KB-per-
    partition descriptor coalescing.
  - L1: per (class, half) DVE max8 gives the top-8 VALUES of each 100-prior
    half per window -- 162 max8 ops, NO find_index8/index tracking. Winner
    prior indices are recovered host-side by exact f32 value search inside
    the statically-known 100-prior half (emulating max8 + stable-tie
    semantics). Window 127 overlaps 126 by 25 priors; the duplicate region
    is NEG-killed via a pre-load memset (32-aligned partition base).
  - box decode runs on the otherwise-idle GpSimd engine (exp on Scalar),
    keeping the DVE stream pure L1+merge.
  - candidates PE-transposed to class-major [81, 2048] (t-major order).
  - 3-tier merge per class, values only:
      C-pool (half-ranks 4..7, 1024 slots) -> top-8
      B-pool (half-ranks 2..3, 512) + C8   -> top-24 (joint B+C winners
                                                      <= 24, verified)
      master = A-pool (half-ranks 0..1, 512) + B24 = 536
    25 rounds of (max8 -> vals slice, match_replace) extract the sorted
    top-200 values. No find_index8 anywhere: the premerge pools (Cval,
    B'val) and the assembled master table are dumped to DRAM; the host
    recovers every position with a stable descending argsort -- a
    bit-exact emulation of the max8/match-replace extraction order.
  - host: master pos -> (window, half) statically, prior index by value
    search, stable-order tie fix-up, box gather from the dumped decode.
"""

import sys

sys.path.insert(0, "/opt/trn_rl_repo")

import numpy as np

import concourse.bass as bass
import concourse.bacc as bacc
import concourse.mybir as mybir
from concourse.bass_types import AP  # noqa: F401
from concourse.masks import make_identity
from concourse.tile import TileContext

F32 = mybir.dt.float32
I32 = mybir.dt.int32
U32 = mybir.dt.uint32

P = 25575            # priors
C = 81               # classes
K = 200              # top-k
NCH = 128            # partitions / prior windows
WIN = 200            # priors per window
HALF = 100           # priors per half-window
NEG = -1.0e30
VAR0, VAR1 = 0.1, 0.2

SLOT = 16            # candidate slots per class per partition
NA, NB, NC_ = 512, 512, 1024   # pool sizes per class
NB2 = NB + 8         # B' = B + C8
NB24 = 24            # B' premerge keep (joint B+C winners <= 24, verified)
NM = NA + NB24       # master size
ROUNDS = 25

FULLP = NCH - 1      # partitions with full windows (127)
TAILI = P - FULLP * WIN   # real priors in the last window (175)
DUPN = WIN - TAILI        # duplicated priors at start of window 127 (25)


def build_nc(compile=True):
    nc = bacc.Bacc()
    conf_in = nc.declare_dram_parameter("conf", [P, C], F32, isOutput=False)
    loc_in = nc.declare_dram_parameter("loc", [P, 4], F32, isOutput=False)
    pri_in = nc.declare_dram_parameter("priors", [P, 4], F32, isOutput=False)
    val_out = nc.declare_dram_parameter("vals", [C, K], F32, isOutput=True)
    m_out = nc.declare_dram_parameter("m0", [C, NM], F32, isOutput=True)
    cv_out = nc.declare_dram_parameter("cval", [C, NC_], F32, isOutput=True)
    bv_out = nc.declare_dram_parameter("bval", [C, NB2], F32, isOutput=True)
    dec_out = nc.declare_dram_parameter("dec", [NCH, WIN * 4], F32, isOutput=True)

    from contextlib import ExitStack

    with TileContext(nc) as tc, ExitStack() as ctx:
        consts = ctx.enter_context(tc.tile_pool(name="consts", bufs=1))
        sb = ctx.enter_context(tc.tile_pool(name="sb", bufs=1))
        psum = ctx.enter_context(tc.tile_pool(name="psum", bufs=2, space="PSUM"))
        small = ctx.enter_context(tc.tile_pool(name="small", bufs=2))

        ident = consts.tile([NCH, NCH], F32)
        make_identity(nc, ident)

        # ---------------- conf load: two half-window tiles -----------------
        # h0 = priors [200p, 200p+100) per window, h1 = [200p+100, 200p+200).
        # Window 127 starts at P-WIN=25375 (overlapping window 126 by 25).
        # The two HWDGE queues (sync / scalar) carry ONLY conf: a dma_start
        # is a blocking DMA_DIRECT2D on the issuing engine, so any small-
        # descriptor DMA queued first would stall the conf load.
        conf_h = []
        full_view = conf_in[: FULLP * WIN, :].rearrange(
            "(p i) c -> p i c", p=FULLP)
        for h in range(2):
            t = sb.tile([NCH, HALF * C], F32, name=f"conf_h{h}")
            conf_h.append(t)
        # kill window-127's duplicated priors [25375, 25400) = first DUPN
        # prior-slices of conf_h0 partition 127. Compute-engine SBUF access
        # needs a 32-aligned partition base, so NEG-fill partitions [96:128)
        # first; the range DMAs below rewrite 96..126 and the h0 tail DMA
        # loads only the real priors [25400, 25475) into cols [DUPN*C:).
        nc.vector.memset(conf_h[0][96:NCH, : DUPN * C], NEG)
        # conf h0 on the sync queue, h1 on the scalar queue: the two HWDGE
        # queues feed disjoint DMA-engine groups, so splitting roughly
        # doubles load bandwidth and h1 lands while L1 h0 still computes.
        # <=64-partition slices keep 32.4KB-per-partition descriptor
        # coalescing (127-partition DMAs shatter into 1.6KB descriptors).
        # h0 split across BOTH queues so it gets all 16 DMA engines first;
        # a DVE gate copy (reads h0, writes a corner of h1's tile, which the
        # h1 DMAs then WAW-wait on) keeps h1 from stealing engine slots
        # until h0 has landed. The gate costs nothing on the DVE: it waits
        # on exactly the same h0 semaphores L1 h0 waits on.
        src_h0 = full_view[:, :HALF, :]
        src_h1 = full_view[:, HALF:, :]
        nc.sync.dma_start(out=conf_h[0][:64, :],
                          in_=src_h0[:64].rearrange("p i c -> p (i c)"))
        nc.scalar.dma_start(out=conf_h[0][64:FULLP, :],
                            in_=src_h0[64:].rearrange("p i c -> p (i c)"))
        nc.scalar.dma_start(
            out=conf_h[0][FULLP:NCH, DUPN * C :],
            in_=conf_in[P - WIN + DUPN : P - WIN + HALF, :]
            .rearrange("(p i) c -> p (i c)", p=1))
        nc.vector.tensor_copy(conf_h[1][:, 0:8], conf_h[0][:, 0:8])
        nc.sync.dma_start(out=conf_h[1][:64, :],
                          in_=src_h1[:64].rearrange("p i c -> p (i c)"))
        nc.scalar.dma_start(out=conf_h[1][64:FULLP, :],
                            in_=src_h1[64:].rearrange("p i c -> p (i c)"))
        nc.scalar.dma_start(
            out=conf_h[1][FULLP:NCH, :],
            in_=conf_in[P - WIN + HALF : P, :]
            .rearrange("(p i) c -> p (i c)", p=1))

        # ---------------- load loc / priors (both queues, after conf) ------
        # descriptor-bound (127 x 3.2KB): queued behind conf so the engine-
        # blocking DMA instructions never delay the conf stream.
        loc_sb = sb.tile([NCH, WIN * 4], F32)
        pri_sb = sb.tile([NCH, WIN * 4], F32)
        # partition 127 reads the OVERLAPPED full window [P-WIN, P); its
        # duplicated priors are neutralized by the conf_h0 memset above.
        for dst, src in ((loc_sb, loc_in), (pri_sb, pri_in)):
            nc.sync.dma_start(
                out=dst[:64, :],
                in_=src[: 64 * WIN, :].rearrange("(p i) c -> p (i c)", p=64),
            )
            nc.scalar.dma_start(
                out=dst[64:FULLP, :],
                in_=src[64 * WIN : FULLP * WIN, :]
                .rearrange("(p i) c -> p (i c)", p=FULLP - 64),
            )
            nc.scalar.dma_start(
                out=dst[FULLP:NCH, :],
                in_=src[P - WIN :, :].rearrange("(p i) c -> p (i c)", p=1),
            )

        # ---------------- L1 h0: per-class top-8 values ---------------------
        # cand_val[p, c*16 + 8h + r] = r-th largest of conf[half h of window p,
        # class c]. No index recovery on device (host does value search).
        cand_val = sb.tile([NCH, C * SLOT], F32)

        def l1_half(h):
            view = conf_h[h][:].rearrange("p (i c) -> p c i", c=C)
            for c in range(C):
                vdst = cand_val[:, c * SLOT + 8 * h : c * SLOT + 8 * h + 8]
                nc.vector.max(vdst, view[:, c, :])

        l1_half(0)
        l1_half(1)

        # ---------------- decode (GpSimd, off the DVE critical path) --------
        def coord(t, k):
            return t[:].rearrange("p (i c) -> p c i", c=4)[:, k, :]

        dec_sb = sb.tile([NCH, WIN * 4], F32)
        cxy = sb.tile([NCH, 2 * WIN], F32)
        wh = sb.tile([NCH, 2 * WIN], F32)
        tmps = [(sb.tile([NCH, WIN], F32, name=f"dtmp1_{k}"),
                 sb.tile([NCH, WIN], F32, name=f"dtmp2_{k}")) for k in range(2)]
        for k in range(2):  # k=0: x, k=1: y
            tmp1, tmp2 = tmps[k]
            Lp, Lwh = coord(loc_sb, k), coord(loc_sb, 2 + k)
            Pp, Pwh = coord(pri_sb, k), coord(pri_sb, 2 + k)
            cx = cxy[:, k * WIN : (k + 1) * WIN]
            w = wh[:, k * WIN : (k + 1) * WIN]
            # w = pw * exp(0.2 * lw)
            nc.gpsimd.tensor_copy(tmp1, Lwh)
            nc.scalar.activation(tmp1, tmp1, mybir.ActivationFunctionType.Exp,
                                 scale=VAR1)
            nc.gpsimd.tensor_mul(w, Pwh, tmp1)
            # cx = px + 0.1 * lx * pw
            nc.gpsimd.tensor_mul(tmp2, Lp, Pwh)
            nc.gpsimd.tensor_scalar_mul(tmp2, tmp2, VAR0)
            nc.gpsimd.tensor_add(cx, Pp, tmp2)
            # x1 = cx - w/2 ; x2 = x1 + w
            nc.gpsimd.tensor_scalar_mul(tmp2, w, 0.5)
            nc.gpsimd.tensor_sub(coord(dec_sb, k), cx, tmp2)
            nc.gpsimd.tensor_add(coord(dec_sb, 2 + k), coord(dec_sb, k), w)
        # dec stored window-flat [128, 800] (one contiguous 3.2KB descriptor
        # per partition); host reshapes. On sync: free after conf.
        nc.sync.dma_start(out=dec_out[:], in_=dec_sb[:])

        # ---------------- transpose candidates to class-major --------------
        val_T = sb.tile([C, NCH * SLOT], F32)
        sview = cand_val[:].rearrange("p (c s) -> p s c", s=SLOT)
        dview = val_T[:].rearrange("q (t s) -> q s t", s=SLOT)
        for grp in (1, 3, 0, 2):
            pt = psum.tile([C, 4 * NCH], F32, tag="tp")
            for k in range(4):
                s = grp * 4 + k
                nc.tensor.transpose(
                    pt[:, k * NCH : (k + 1) * NCH], sview[:, s, :], ident[:]
                )
            nc.scalar.copy(
                dview[:, grp * 4 : grp * 4 + 4, :],
                pt[:].rearrange("q (k t) -> q k t", k=4),
            )

        # t-major slot views: A: s in {0,1,8,9}, B: {2,3,10,11}, C: {4..7,12..15}
        def pool_view(t, s0):
            return t[:].rearrange("q (t h s) -> q t h s", h=2, s=8)[
                :, :, :, s0 : s0 + 2
            ]

        def poolC_view(t):
            return t[:].rearrange("q (t h s) -> q t h s", h=2, s=8)[:, :, :, 4:8]

        # ---------------- C-pool premerge: top-8 of 1024 --------------------
        # positions recovered host-side by stable argsort of the dumped pool
        # (same emulation as the master table), so no find_index8 anywhere.
        Cval = sb.tile([C, NC_], F32)
        nc.scalar.copy(Cval[:].rearrange("q (t h s) -> q t h s", h=2, s=4),
                       poolC_view(val_T))
        nc.sync.dma_start(out=cv_out[:], in_=Cval[:])
        c8val = small.tile([C, 8], F32, tag="c8v")
        nc.vector.max(c8val, Cval)

        # ---------------- B' = B + C8 premerge: top-24 ----------------------
        Bval = sb.tile([C, NB2], F32)
        nc.scalar.copy(Bval[:, :NB].rearrange("q (t h s) -> q t h s", h=2, s=2),
                       pool_view(val_T, 2))
        nc.vector.tensor_copy(Bval[:, NB:NB2], c8val)
        Bdump = sb.tile([C, NB2], F32)
        nc.scalar.copy(Bdump[:], Bval[:])
        nc.sync.dma_start(out=bv_out[:], in_=Bdump[:])

        b24val = sb.tile([C, NB24], F32)
        for r in range(3):
            vs = b24val[:, 8 * r : 8 * r + 8]
            nc.vector.max(vs, Bval)
            if r < 2:
                nc.vector.match_replace(Bval, vs, Bval, NEG)

        # ---------------- master = A + B24 ----------------------------------
        Mval = sb.tile([C, NM], F32)
        nc.scalar.copy(Mval[:, :NA].rearrange("q (t h s) -> q t h s", h=2, s=2),
                       pool_view(val_T, 0))
        nc.vector.tensor_copy(Mval[:, NA:NM], b24val)

        # dump the assembled master table (positions recovered host-side by
        # stable argsort -- exactly the max8/match-replace extraction order).
        # Copy first so round-1's match_replace doesn't wait on the DMA.
        Mdump = sb.tile([C, NM], F32)
        nc.scalar.copy(Mdump[:], Mval[:])
        nc.sync.dma_start(out=m_out[:], in_=Mdump[:])

        # ---------------- 25 extraction rounds (values only) ----------------
        # max8 writes straight into the vals slice; match_replace reads its
        # needles from the same slice -- no staging, no cross-engine traffic.
        vals_sb = sb.tile([C, K], F32)
        for r in range(ROUNDS):
            vs = vals_sb[:, 8 * r : 8 * r + 8]
            nc.vector.max(vs, Mval)
            nc.vector.match_replace(Mval, vs, Mval, NEG)
            if r == ROUNDS - 3:
                # rounds 0..22 done: ship the first 184 columns while the
                # last two rounds run, partition-split across both queues
                nc.sync.dma_start(out=val_out[: C // 2, : 8 * (ROUNDS - 2)],
                                  in_=vals_sb[: C // 2, : 8 * (ROUNDS - 2)])
                nc.scalar.dma_start(
                    out=val_out[C // 2 :, : 8 * (ROUNDS - 2)],
                    in_=vals_sb[C // 2 :, : 8 * (ROUNDS - 2)])

        # last 16 columns: split across both queues to halve descriptor tail
        nc.sync.dma_start(out=val_out[: C // 2, 8 * (ROUNDS - 2) :],
                          in_=vals_sb[: C // 2, 8 * (ROUNDS - 2) :])
        nc.scalar.dma_start(out=val_out[C // 2 :, 8 * (ROUNDS - 2) :],
                            in_=vals_sb[C // 2 :, 8 * (ROUNDS - 2) :])

    if compile:
        nc.compile()
    return nc


_NC = None


def _get_nc():
    global _NC
    if _NC is None:
        _NC = build_nc()
    return _NC


def _install_ntff_shim():
    """The container's antenv lacks axon_hooks; synthesize it from the boot
    module's ctypes NTFF driver so trace=True can profile."""
    import types

    if "antenv.axon_hooks" in sys.modules:
        return
    try:
        from trn_agent_boot.trn_boot import _ntff_profile_via_ctypes

        hook = _ntff_profile_via_ctypes("/opt/axon/libaxon_pjrt.so")
    except Exception:
        hook = None
    mod = types.ModuleType("antenv.axon_hooks")
    mod._hook = hook
    mod.get_axon_ntff_profile_hook = lambda: mod._hook
    mod.set_axon_ntff_profile_hook = lambda h: setattr(mod, "_hook", h)
    sys.modules["antenv.axon_hooks"] = mod


def _decode_master_pos(qbuf, c8pos, b24pos):
    """Map master positions [C, K] -> (window t, half h) per winner.

    Master layout: pos < 512 -> A-pool (t*4 + h*2 + rank); pos >= 512 ->
    b24pos[pos-512] -> B' pool: < 512 -> B (t*4 + h*2 + rank), >= 512 ->
    c8pos[.-512] -> C-pool (t*8 + h*4 + rank).
    """
    m = qbuf.astype(np.int64)                      # [C, K]
    t = np.empty_like(m)
    hh = np.empty_like(m)

    inA = m < NA
    t[inA] = m[inA] // 4
    hh[inA] = (m[inA] // 2) % 2

    j = np.clip(m - NA, 0, NB24 - 1)
    p = np.take_along_axis(b24pos.astype(np.int64), j, axis=1)  # [C, K]
    inB = (~inA) & (p < NB)
    t[inB] = p[inB] // 4
    hh[inB] = (p[inB] // 2) % 2

    q = np.take_along_axis(c8pos.astype(np.int64), np.clip(p - NB, 0, 7), axis=1)
    inC = (~inA) & (p >= NB)
    t[inC] = q[inC] // 8
    hh[inC] = (q[inC] // 4) % 2
    return t, hh


def _resolve_prior_indices(conf_b, vals, t, hh):
    """Resolve each winner's prior index by exact value search in its
    100-prior half-window (emulating max8 + stable-tie semantics)."""
    Cn, Kn = vals.shape
    tf = t.ravel()
    hf = hh.ravel()
    cf = np.repeat(np.arange(Cn), Kn)
    vf = vals.ravel()

    # search domain [lo, hi): window start 200t (window 127 starts at 25375),
    # but window 127 half 0's first DUPN priors were killed on device.
    ws = np.where(tf < FULLP, WIN * tf, P - WIN)
    lo = ws + HALF * hf
    hi = lo + HALF
    t127h0 = (tf == FULLP) & (hf == 0)
    lo = np.where(t127h0, P - WIN + DUPN, lo)

    idx = lo[:, None] + np.arange(HALF)[None, :]          # [N, 100]
    idx_c = np.minimum(idx, P - 1)
    S = conf_b[idx_c, cf[:, None]]                        # gathered slices
    eq = (S == vf[:, None]) & (idx < hi[:, None])
    am = eq.argmax(axis=1)
    gidx = lo + am

    # duplicate winners in the same (c, lo) with the same value: assign
    # successive occurrences in output-rank order (stable)
    key = np.stack([cf, lo, vf.view(np.int32).astype(np.int64)], axis=1)
    _, inv, counts = np.unique(key, axis=0, return_inverse=True,
                               return_counts=True)
    dup_groups = np.flatnonzero(counts > 1)
    if dup_groups.size:
        for g in dup_groups:
            rows = np.flatnonzero(inv == g)               # in rank order
            occ = np.flatnonzero(eq[rows[0]])
            n = min(len(rows), len(occ))
            gidx[rows[:n]] = lo[rows[0]] + occ[:n]
    return gidx.reshape(Cn, Kn)


def _run(loc_data, conf_data, prior_data, trace=False):
    from concourse.bass_utils import run_bass_kernel_spmd

    if trace:
        _install_ntff_shim()

    nc = _get_nc()
    B = conf_data.shape[0]
    in_maps = [
        {
            "conf": np.ascontiguousarray(conf_data[b], dtype=np.float32),
            "loc": np.ascontiguousarray(loc_data[b], dtype=np.float32),
            "priors": np.ascontiguousarray(prior_data[0], dtype=np.float32),
        }
        for b in range(B)
    ]
    res = run_bass_kernel_spmd(nc, in_maps, list(range(B)), trace=trace)
    out = np.empty((B, C, K, 5), np.float32)
    for b in range(B):
        r = res.results[b]
        vals = np.asarray(r["vals"])              # [C, K] sorted desc
        m0 = np.asarray(r["m0"])                  # [C, NM] master table
        cval = np.asarray(r["cval"])              # [C, 1024] C pool
        bval = np.asarray(r["bval"])              # [C, 520]  B' pool
        # device extraction == stable descending order of each table
        qbuf = np.argsort(-m0, axis=1, kind="stable")[:, :K].astype(np.uint32)
        c8pos = np.argsort(-cval, axis=1, kind="stable")[:, :8]
        b24pos = np.argsort(-bval, axis=1, kind="stable")[:, :NB24]
        dec_flat = np.asarray(r["dec"]).reshape(NCH, WIN, 4)
        dec = np.concatenate(
            [dec_flat[:FULLP].reshape(-1, 4), dec_flat[FULLP, DUPN:]])
        t, hh = _decode_master_pos(qbuf, c8pos, b24pos)
        gidx = _resolve_prior_indices(in_maps[b]["conf"], vals, t, hh)
        # stable-order repair: adjacent equal values whose prior order is
        # inverted (cross-pool ties) are swapped to match jax.lax.top_k
        eq = vals[:, :-1] == vals[:, 1:]
        gt = gidx[:, :-1] > gidx[:, 1:]
        sw = np.where(eq & gt)
        l, rr = sw[0], sw[1]
        g2 = gidx.copy()
        g2[l, rr], g2[l, rr + 1] = gidx[l, rr + 1], gidx[l, rr]
        out[b, :, :, 0] = vals
        out[b, :, :, 1:] = dec[g2]
    return out, res


def kernel(loc_data, conf_data, prior_data):
    out, _ = _run(np.asarray(loc_data), np.asarray(conf_data),
                  np.asarray(prior_data))
    return out


# revision 33
# speedup vs baseline: 1.0219x; 1.0219x over previous
"""SSD Detect (decode + per-class top-200) Trainium2 Bass kernel.

Sharding: data-parallel over batch. 8 batches -> 8 NeuronCores, one batch per
core. Each core computes, for its batch:
  decoded boxes [25575, 4]  (SSD decode from loc + priors)
  per class c in [0, 81): top-200 scores (desc, ties -> lower prior index
  first, matching jax.lax.top_k) with their decoded boxes ->
  out[c, r] = [score_r, x1, y1, x2, y2]

Device algorithm per core:
  - conf is loaded as TWO half-window tiles (priors [200p, 200p+100) and
    [200p+100, 200p+200) per window), each half split across BOTH HWDGE
    queues so it uses all 16 DMA engines (~190GB/s). A DVE gate copy makes
    the h1 DMAs WAW-wait for h0, so h0 lands at ~30us with full bandwidth
    and L1 h0 overlaps the h1 load. Descriptor-heavy transfers (loc/priors:
    127 x 3.2KB; dma_start is a blocking DMA_DIRECT2D on its issuing
    engine) are queued after conf. <=64-partition slices keep 32.4KB-per-
    partition descriptor coalescing.
  - L1: per (class, half) DVE max8 gives the top-8 VALUES of each 100-prior
    half per window -- 162 max8 ops, NO find_index8/index tracking. Winner
    prior indices are recovered host-side by exact f32 value search inside
    the statically-known 100-prior half (emulating max8 + stable-tie
    semantics). Window 127 overlaps 126 by 25 priors; the duplicate region
    is NEG-killed via a pre-load memset (32-aligned partition base).
  - box decode runs on the otherwise-idle GpSimd engine (exp on Scalar),
    keeping the DVE stream pure L1+merge.
  - candidates PE-transposed to class-major [81, 2048] (t-major order).
  - 3-tier merge per class, values only:
      C-pool (half-ranks 4..7, 1024 slots) -> top-8
      B-pool (half-ranks 2..3, 512) + C8   -> top-24 (joint B+C winners
                                                      <= 24, verified)
      master = A-pool (half-ranks 0..1, 512) + B24 = 536
    25 rounds of (max8 -> vals slice, match_replace) extract the sorted
    top-200 values. No find_index8 anywhere: the premerge pools (Cval,
    B'val) and the assembled master table are dumped to DRAM; the host
    recovers every position with a stable descending argsort -- a
    bit-exact emulation of the max8/match-replace extraction order.
  - host: master pos -> (window, half) statically, prior index by value
    search, stable-order tie fix-up, box gather from the dumped decode.
"""

import sys

sys.path.insert(0, "/opt/trn_rl_repo")

import numpy as np

import concourse.bass as bass
import concourse.bacc as bacc
import concourse.mybir as mybir
from concourse.bass_types import AP  # noqa: F401
from concourse.masks import make_identity
from concourse.tile import TileContext

F32 = mybir.dt.float32
I32 = mybir.dt.int32
U32 = mybir.dt.uint32

P = 25575            # priors
C = 81               # classes
K = 200              # top-k
NCH = 128            # partitions / prior windows
WIN = 200            # priors per window
HALF = 100           # priors per half-window
NEG = -1.0e30
VAR0, VAR1 = 0.1, 0.2

SLOT = 16            # candidate slots per class per partition
NA, NB, NC_ = 512, 512, 1024   # pool sizes per class
NB2 = NB + 8         # B' = B + C8
NB24 = 24            # B' premerge keep (joint B+C winners <= 24, verified)
NM = NA + NB24       # master size
ROUNDS = 25

FULLP = NCH - 1      # partitions with full windows (127)
TAILI = P - FULLP * WIN   # real priors in the last window (175)
DUPN = WIN - TAILI        # duplicated priors at start of window 127 (25)


def build_nc(compile=True):
    nc = bacc.Bacc()
    conf_in = nc.declare_dram_parameter("conf", [P, C], F32, isOutput=False)
    loc_in = nc.declare_dram_parameter("loc", [P, 4], F32, isOutput=False)
    pri_in = nc.declare_dram_parameter("priors", [P, 4], F32, isOutput=False)
    val_out = nc.declare_dram_parameter("vals", [C, K], F32, isOutput=True)
    m_out = nc.declare_dram_parameter("m0", [C, NM], F32, isOutput=True)
    cv_out = nc.declare_dram_parameter("cval", [C, NC_], F32, isOutput=True)
    bv_out = nc.declare_dram_parameter("bval", [C, NB2], F32, isOutput=True)
    dec_out = nc.declare_dram_parameter("dec", [NCH, WIN * 4], F32, isOutput=True)

    from contextlib import ExitStack

    with TileContext(nc) as tc, ExitStack() as ctx:
        consts = ctx.enter_context(tc.tile_pool(name="consts", bufs=1))
        sb = ctx.enter_context(tc.tile_pool(name="sb", bufs=1))
        psum = ctx.enter_context(tc.tile_pool(name="psum", bufs=2, space="PSUM"))
        small = ctx.enter_context(tc.tile_pool(name="small", bufs=2))

        ident = consts.tile([NCH, NCH], F32)
        make_identity(nc, ident)

        # ---------------- conf load: two half-window tiles -----------------
        # h0 = priors [200p, 200p+100) per window, h1 = [200p+100, 200p+200).
        # Window 127 starts at P-WIN=25375 (overlapping window 126 by 25).
        # The two HWDGE queues (sync / scalar) carry ONLY conf: a dma_start
        # is a blocking DMA_DIRECT2D on the issuing engine, so any small-
        # descriptor DMA queued first would stall the conf load.
        conf_h = []
        full_view = conf_in[: FULLP * WIN, :].rearrange(
            "(p i) c -> p i c", p=FULLP)
        for h in range(2):
            t = sb.tile([NCH, HALF * C], F32, name=f"conf_h{h}")
            conf_h.append(t)
        # kill window-127's duplicated priors [25375, 25400) = first DUPN
        # prior-slices of conf_h0 partition 127. Compute-engine SBUF access
        # needs a 32-aligned partition base, so NEG-fill partitions [96:128)
        # first; the range DMAs below rewrite 96..126 and the h0 tail DMA
        # loads only the real priors [25400, 25475) into cols [DUPN*C:).
        nc.vector.memset(conf_h[0][96:NCH, : DUPN * C], NEG)
        # conf h0 on the sync queue, h1 on the scalar queue: the two HWDGE
        # queues feed disjoint DMA-engine groups, so splitting roughly
        # doubles load bandwidth and h1 lands while L1 h0 still computes.
        # <=64-partition slices keep 32.4KB-per-partition descriptor
        # coalescing (127-partition DMAs shatter into 1.6KB descriptors).
        # h0 split across BOTH queues so it gets all 16 DMA engines first;
        # a DVE gate copy (reads h0, writes a corner of h1's tile, which the
        # h1 DMAs then WAW-wait on) keeps h1 from stealing engine slots
        # until h0 has landed. The gate costs nothing on the DVE: it waits
        # on exactly the same h0 semaphores L1 h0 waits on.
        src_h0 = full_view[:, :HALF, :]
        src_h1 = full_view[:, HALF:, :]
        nc.sync.dma_start(out=conf_h[0][:64, :],
                          in_=src_h0[:64].rearrange("p i c -> p (i c)"))
        nc.scalar.dma_start(out=conf_h[0][64:FULLP, :],
                            in_=src_h0[64:].rearrange("p i c -> p (i c)"))
        nc.scalar.dma_start(
            out=conf_h[0][FULLP:NCH, DUPN * C :],
            in_=conf_in[P - WIN + DUPN : P - WIN + HALF, :]
            .rearrange("(p i) c -> p (i c)", p=1))
        nc.vector.tensor_copy(conf_h[1][:, 0:8], conf_h[0][:, 0:8])
        nc.sync.dma_start(out=conf_h[1][:64, :],
                          in_=src_h1[:64].rearrange("p i c -> p (i c)"))
        nc.scalar.dma_start(out=conf_h[1][64:FULLP, :],
                            in_=src_h1[64:].rearrange("p i c -> p (i c)"))
        nc.scalar.dma_start(
            out=conf_h[1][FULLP:NCH, :],
            in_=conf_in[P - WIN + HALF : P, :]
            .rearrange("(p i) c -> p (i c)", p=1))

        # ---------------- load loc / priors (both queues, after conf) ------
        # descriptor-bound (127 x 3.2KB): queued behind conf so the engine-
        # blocking DMA instructions never delay the conf stream.
        loc_sb = sb.tile([NCH, WIN * 4], F32)
        pri_sb = sb.tile([NCH, WIN * 4], F32)
        # gates: loc/pri descriptors would steal ~4 of 16 DMA engines from
        # the conf h1 stream; these GpSimd copies (overwritten by the loads)
        # make the loc/pri DMAs WAW-wait until h1 has landed.
        nc.gpsimd.tensor_copy(loc_sb[:, 0:64], conf_h[1][:, 0:64])
        nc.gpsimd.tensor_copy(pri_sb[:, 0:64], conf_h[1][:, 0:64])
        # partition 127 reads the OVERLAPPED full window [P-WIN, P); its
        # duplicated priors are neutralized by the conf_h0 memset above.
        for dst, src in ((loc_sb, loc_in), (pri_sb, pri_in)):
            nc.sync.dma_start(
                out=dst[:64, :],
                in_=src[: 64 * WIN, :].rearrange("(p i) c -> p (i c)", p=64),
            )
            nc.scalar.dma_start(
                out=dst[64:FULLP, :],
                in_=src[64 * WIN : FULLP * WIN, :]
                .rearrange("(p i) c -> p (i c)", p=FULLP - 64),
            )
            nc.scalar.dma_start(
                out=dst[FULLP:NCH, :],
                in_=src[P - WIN :, :].rearrange("(p i) c -> p (i c)", p=1),
            )

        # ---------------- L1 h0: per-class top-8 values ---------------------
        # cand_val[p, c*16 + 8h + r] = r-th largest of conf[half h of window p,
        # class c]. No index recovery on device (host does value search).
        cand_val = sb.tile([NCH, C * SLOT], F32)

        def l1_half(h):
            view = conf_h[h][:].rearrange("p (i c) -> p c i", c=C)
            for c in range(C):
                vdst = cand_val[:, c * SLOT + 8 * h : c * SLOT + 8 * h + 8]
                nc.vector.max(vdst, view[:, c, :])

        l1_half(0)
        l1_half(1)

        # ---------------- decode (GpSimd, off the DVE critical path) --------
        def coord(t, k):
            return t[:].rearrange("p (i c) -> p c i", c=4)[:, k, :]

        dec_sb = sb.tile([NCH, WIN * 4], F32)
        cxy = sb.tile([NCH, 2 * WIN], F32)
        wh = sb.tile([NCH, 2 * WIN], F32)
        tmps = [(sb.tile([NCH, WIN], F32, name=f"dtmp1_{k}"),
                 sb.tile([NCH, WIN], F32, name=f"dtmp2_{k}")) for k in range(2)]
        for k in range(2):  # k=0: x, k=1: y
            tmp1, tmp2 = tmps[k]
            Lp, Lwh = coord(loc_sb, k), coord(loc_sb, 2 + k)
            Pp, Pwh = coord(pri_sb, k), coord(pri_sb, 2 + k)
            cx = cxy[:, k * WIN : (k + 1) * WIN]
            w = wh[:, k * WIN : (k + 1) * WIN]
            # w = pw * exp(0.2 * lw)
            nc.gpsimd.tensor_copy(tmp1, Lwh)
            nc.scalar.activation(tmp1, tmp1, mybir.ActivationFunctionType.Exp,
                                 scale=VAR1)
            nc.gpsimd.tensor_mul(w, Pwh, tmp1)
            # cx = px + 0.1 * lx * pw
            nc.gpsimd.tensor_mul(tmp2, Lp, Pwh)
            nc.gpsimd.tensor_scalar_mul(tmp2, tmp2, VAR0)
            nc.gpsimd.tensor_add(cx, Pp, tmp2)
            # x1 = cx - w/2 ; x2 = x1 + w
            nc.gpsimd.tensor_scalar_mul(tmp2, w, 0.5)
            nc.gpsimd.tensor_sub(coord(dec_sb, k), cx, tmp2)
            nc.gpsimd.tensor_add(coord(dec_sb, 2 + k), coord(dec_sb, k), w)
        # dec stored window-flat [128, 800] (one contiguous 3.2KB descriptor
        # per partition); host reshapes. On sync: free after conf.
        nc.sync.dma_start(out=dec_out[:], in_=dec_sb[:])

        # ---------------- transpose candidates to class-major --------------
        val_T = sb.tile([C, NCH * SLOT], F32)
        sview = cand_val[:].rearrange("p (c s) -> p s c", s=SLOT)
        dview = val_T[:].rearrange("q (t s) -> q s t", s=SLOT)
        for grp in (1, 3, 0, 2):
            pt = psum.tile([C, 4 * NCH], F32, tag="tp")
            for k in range(4):
                s = grp * 4 + k
                nc.tensor.transpose(
                    pt[:, k * NCH : (k + 1) * NCH], sview[:, s, :], ident[:]
                )
            nc.scalar.copy(
                dview[:, grp * 4 : grp * 4 + 4, :],
                pt[:].rearrange("q (k t) -> q k t", k=4),
            )

        # t-major slot views: A: s in {0,1,8,9}, B: {2,3,10,11}, C: {4..7,12..15}
        def pool_view(t, s0):
            return t[:].rearrange("q (t h s) -> q t h s", h=2, s=8)[
                :, :, :, s0 : s0 + 2
            ]

        def poolC_view(t):
            return t[:].rearrange("q (t h s) -> q t h s", h=2, s=8)[:, :, :, 4:8]

        # ---------------- C-pool premerge: top-8 of 1024 --------------------
        # positions recovered host-side by stable argsort of the dumped pool
        # (same emulation as the master table), so no find_index8 anywhere.
        Cval = sb.tile([C, NC_], F32)
        nc.scalar.copy(Cval[:].rearrange("q (t h s) -> q t h s", h=2, s=4),
                       poolC_view(val_T))
        nc.sync.dma_start(out=cv_out[:], in_=Cval[:])
        c8val = small.tile([C, 8], F32, tag="c8v")
        nc.vector.max(c8val, Cval)

        # ---------------- B' = B + C8 premerge: top-24 ----------------------
        Bval = sb.tile([C, NB2], F32)
        nc.scalar.copy(Bval[:, :NB].rearrange("q (t h s) -> q t h s", h=2, s=2),
                       pool_view(val_T, 2))
        nc.vector.tensor_copy(Bval[:, NB:NB2], c8val)
        Bdump = sb.tile([C, NB2], F32)
        nc.scalar.copy(Bdump[:], Bval[:])
        nc.sync.dma_start(out=bv_out[:], in_=Bdump[:])

        b24val = sb.tile([C, NB24], F32)
        for r in range(3):
            vs = b24val[:, 8 * r : 8 * r + 8]
            nc.vector.max(vs, Bval)
            if r < 2:
                nc.vector.match_replace(Bval, vs, Bval, NEG)

        # ---------------- master = A + B24 ----------------------------------
        Mval = sb.tile([C, NM], F32)
        nc.scalar.copy(Mval[:, :NA].rearrange("q (t h s) -> q t h s", h=2, s=2),
                       pool_view(val_T, 0))
        nc.vector.tensor_copy(Mval[:, NA:NM], b24val)

        # dump the assembled master table (positions recovered host-side by
        # stable argsort -- exactly the max8/match-replace extraction order).
        # Copy first so round-1's match_replace doesn't wait on the DMA.
        Mdump = sb.tile([C, NM], F32)
        nc.scalar.copy(Mdump[:], Mval[:])
        nc.sync.dma_start(out=m_out[:], in_=Mdump[:])

        # ---------------- 25 extraction rounds (values only) ----------------
        # max8 writes straight into the vals slice; match_replace reads its
        # needles from the same slice -- no staging, no cross-engine traffic.
        vals_sb = sb.tile([C, K], F32)
        for r in range(ROUNDS):
            vs = vals_sb[:, 8 * r : 8 * r + 8]
            nc.vector.max(vs, Mval)
            nc.vector.match_replace(Mval, vs, Mval, NEG)
            if r == ROUNDS - 3:
                # rounds 0..22 done: ship the first 184 columns while the
                # last two rounds run, partition-split across both queues
                nc.sync.dma_start(out=val_out[: C // 2, : 8 * (ROUNDS - 2)],
                                  in_=vals_sb[: C // 2, : 8 * (ROUNDS - 2)])
                nc.scalar.dma_start(
                    out=val_out[C // 2 :, : 8 * (ROUNDS - 2)],
                    in_=vals_sb[C // 2 :, : 8 * (ROUNDS - 2)])

        # last 16 columns: split across both queues to halve descriptor tail
        nc.sync.dma_start(out=val_out[: C // 2, 8 * (ROUNDS - 2) :],
                          in_=vals_sb[: C // 2, 8 * (ROUNDS - 2) :])
        nc.scalar.dma_start(out=val_out[C // 2 :, 8 * (ROUNDS - 2) :],
                            in_=vals_sb[C // 2 :, 8 * (ROUNDS - 2) :])

    if compile:
        nc.compile()
    return nc


_NC = None


def _get_nc():
    global _NC
    if _NC is None:
        _NC = build_nc()
    return _NC


def _install_ntff_shim():
    """The container's antenv lacks axon_hooks; synthesize it from the boot
    module's ctypes NTFF driver so trace=True can profile."""
    import types

    if "antenv.axon_hooks" in sys.modules:
        return
    try:
        from trn_agent_boot.trn_boot import _ntff_profile_via_ctypes

        hook = _ntff_profile_via_ctypes("/opt/axon/libaxon_pjrt.so")
    except Exception:
        hook = None
    mod = types.ModuleType("antenv.axon_hooks")
    mod._hook = hook
    mod.get_axon_ntff_profile_hook = lambda: mod._hook
    mod.set_axon_ntff_profile_hook = lambda h: setattr(mod, "_hook", h)
    sys.modules["antenv.axon_hooks"] = mod


def _decode_master_pos(qbuf, c8pos, b24pos):
    """Map master positions [C, K] -> (window t, half h) per winner.

    Master layout: pos < 512 -> A-pool (t*4 + h*2 + rank); pos >= 512 ->
    b24pos[pos-512] -> B' pool: < 512 -> B (t*4 + h*2 + rank), >= 512 ->
    c8pos[.-512] -> C-pool (t*8 + h*4 + rank).
    """
    m = qbuf.astype(np.int64)                      # [C, K]
    t = np.empty_like(m)
    hh = np.empty_like(m)

    inA = m < NA
    t[inA] = m[inA] // 4
    hh[inA] = (m[inA] // 2) % 2

    j = np.clip(m - NA, 0, NB24 - 1)
    p = np.take_along_axis(b24pos.astype(np.int64), j, axis=1)  # [C, K]
    inB = (~inA) & (p < NB)
    t[inB] = p[inB] // 4
    hh[inB] = (p[inB] // 2) % 2

    q = np.take_along_axis(c8pos.astype(np.int64), np.clip(p - NB, 0, 7), axis=1)
    inC = (~inA) & (p >= NB)
    t[inC] = q[inC] // 8
    hh[inC] = (q[inC] // 4) % 2
    return t, hh


def _resolve_prior_indices(conf_b, vals, t, hh):
    """Resolve each winner's prior index by exact value search in its
    100-prior half-window (emulating max8 + stable-tie semantics)."""
    Cn, Kn = vals.shape
    tf = t.ravel()
    hf = hh.ravel()
    cf = np.repeat(np.arange(Cn), Kn)
    vf = vals.ravel()

    # search domain [lo, hi): window start 200t (window 127 starts at 25375),
    # but window 127 half 0's first DUPN priors were killed on device.
    ws = np.where(tf < FULLP, WIN * tf, P - WIN)
    lo = ws + HALF * hf
    hi = lo + HALF
    t127h0 = (tf == FULLP) & (hf == 0)
    lo = np.where(t127h0, P - WIN + DUPN, lo)

    idx = lo[:, None] + np.arange(HALF)[None, :]          # [N, 100]
    idx_c = np.minimum(idx, P - 1)
    S = conf_b[idx_c, cf[:, None]]                        # gathered slices
    eq = (S == vf[:, None]) & (idx < hi[:, None])
    am = eq.argmax(axis=1)
    gidx = lo + am

    # duplicate winners in the same (c, lo) with the same value: assign
    # successive occurrences in output-rank order (stable)
    key = np.stack([cf, lo, vf.view(np.int32).astype(np.int64)], axis=1)
    _, inv, counts = np.unique(key, axis=0, return_inverse=True,
                               return_counts=True)
    dup_groups = np.flatnonzero(counts > 1)
    if dup_groups.size:
        for g in dup_groups:
            rows = np.flatnonzero(inv == g)               # in rank order
            occ = np.flatnonzero(eq[rows[0]])
            n = min(len(rows), len(occ))
            gidx[rows[:n]] = lo[rows[0]] + occ[:n]
    return gidx.reshape(Cn, Kn)


def _run(loc_data, conf_data, prior_data, trace=False):
    from concourse.bass_utils import run_bass_kernel_spmd

    if trace:
        _install_ntff_shim()

    nc = _get_nc()
    B = conf_data.shape[0]
    in_maps = [
        {
            "conf": np.ascontiguousarray(conf_data[b], dtype=np.float32),
            "loc": np.ascontiguousarray(loc_data[b], dtype=np.float32),
            "priors": np.ascontiguousarray(prior_data[0], dtype=np.float32),
        }
        for b in range(B)
    ]
    res = run_bass_kernel_spmd(nc, in_maps, list(range(B)), trace=trace)
    out = np.empty((B, C, K, 5), np.float32)
    for b in range(B):
        r = res.results[b]
        vals = np.asarray(r["vals"])              # [C, K] sorted desc
        m0 = np.asarray(r["m0"])                  # [C, NM] master table
        cval = np.asarray(r["cval"])              # [C, 1024] C pool
        bval = np.asarray(r["bval"])              # [C, 520]  B' pool
        # device extraction == stable descending order of each table
        qbuf = np.argsort(-m0, axis=1, kind="stable")[:, :K].astype(np.uint32)
        c8pos = np.argsort(-cval, axis=1, kind="stable")[:, :8]
        b24pos = np.argsort(-bval, axis=1, kind="stable")[:, :NB24]
        dec_flat = np.asarray(r["dec"]).reshape(NCH, WIN, 4)
        dec = np.concatenate(
            [dec_flat[:FULLP].reshape(-1, 4), dec_flat[FULLP, DUPN:]])
        t, hh = _decode_master_pos(qbuf, c8pos, b24pos)
        gidx = _resolve_prior_indices(in_maps[b]["conf"], vals, t, hh)
        # stable-order repair: adjacent equal values whose prior order is
        # inverted (cross-pool ties) are swapped to match jax.lax.top_k
        eq = vals[:, :-1] == vals[:, 1:]
        gt = gidx[:, :-1] > gidx[:, 1:]
        sw = np.where(eq & gt)
        l, rr = sw[0], sw[1]
        g2 = gidx.copy()
        g2[l, rr], g2[l, rr + 1] = gidx[l, rr + 1], gidx[l, rr]
        out[b, :, :, 0] = vals
        out[b, :, :, 1:] = dec[g2]
    return out, res


def kernel(loc_data, conf_data, prior_data):
    out, _ = _run(np.asarray(loc_data), np.asarray(conf_data),
                  np.asarray(prior_data))
    return out


# revision 34
# speedup vs baseline: 1.0282x; 1.0062x over previous
"""SSD Detect (decode + per-class top-200) Trainium2 Bass kernel.

Sharding: data-parallel over batch. 8 batches -> 8 NeuronCores, one batch per
core. Each core computes, for its batch:
  decoded boxes [25575, 4]  (SSD decode from loc + priors)
  per class c in [0, 81): top-200 scores (desc, ties -> lower prior index
  first, matching jax.lax.top_k) with their decoded boxes ->
  out[c, r] = [score_r, x1, y1, x2, y2]

Device algorithm per core:
  - conf is loaded as TWO half-window tiles (priors [200p, 200p+100) and
    [200p+100, 200p+200) per window), each half split across BOTH HWDGE
    queues so it uses all 16 DMA engines (~190GB/s). A DVE gate copy makes
    the h1 DMAs WAW-wait for h0, so h0 lands at ~30us with full bandwidth
    and L1 h0 overlaps the h1 load. Descriptor-heavy transfers (loc/priors:
    127 x 3.2KB; dma_start is a blocking DMA_DIRECT2D on its issuing
    engine) are queued after conf. <=64-partition slices keep 32.4KB-per-
    partition descriptor coalescing.
  - L1: per (class, half) DVE max8 gives the top-8 VALUES of each 100-prior
    half per window -- 162 max8 ops, NO find_index8/index tracking. Winner
    prior indices are recovered host-side by exact f32 value search inside
    the statically-known 100-prior half (emulating max8 + stable-tie
    semantics). Window 127 overlaps 126 by 25 priors; the duplicate region
    is NEG-killed via a pre-load memset (32-aligned partition base).
  - box decode runs on the otherwise-idle GpSimd engine (exp on Scalar),
    keeping the DVE stream pure L1+merge.
  - candidates PE-transposed to class-major [81, 2048] (t-major order).
  - 3-tier merge per class, values only:
      C-pool (half-ranks 4..7, 1024 slots) -> top-8
      B-pool (half-ranks 2..3, 512) + C8   -> top-24 (joint B+C winners
                                                      <= 24, verified)
      master = A-pool (half-ranks 0..1, 512) + B24 = 536
    25 rounds of (max8 -> vals slice, match_replace) extract the sorted
    top-200 values. No find_index8 anywhere: the premerge pools (Cval,
    B'val) and the assembled master table are dumped to DRAM; the host
    recovers every position with a stable descending argsort -- a
    bit-exact emulation of the max8/match-replace extraction order.
  - host: master pos -> (window, half) statically, prior index by value
    search, stable-order tie fix-up, box gather from the dumped decode.
"""

import sys

sys.path.insert(0, "/opt/trn_rl_repo")

import numpy as np

import concourse.bass as bass
import concourse.bacc as bacc
import concourse.mybir as mybir
from concourse.bass_types import AP  # noqa: F401
from concourse.masks import make_identity
from concourse.tile import TileContext

F32 = mybir.dt.float32
I32 = mybir.dt.int32
U32 = mybir.dt.uint32

P = 25575            # priors
C = 81               # classes
K = 200              # top-k
NCH = 128            # partitions / prior windows
WIN = 200            # priors per window
HALF = 100           # priors per half-window
NEG = -1.0e30
VAR0, VAR1 = 0.1, 0.2

SLOT = 16            # candidate slots per class per partition
NA, NB, NC_ = 512, 512, 1024   # pool sizes per class
NB2 = NB + 8         # B' = B + C8
NB24 = 24            # B' premerge keep (joint B+C winners <= 24, verified)
NM = NA + NB24       # master size
ROUNDS = 25

FULLP = NCH - 1      # partitions with full windows (127)
TAILI = P - FULLP * WIN   # real priors in the last window (175)
DUPN = WIN - TAILI        # duplicated priors at start of window 127 (25)


def build_nc(compile=True):
    nc = bacc.Bacc()
    conf_in = nc.declare_dram_parameter("conf", [P, C], F32, isOutput=False)
    loc_in = nc.declare_dram_parameter("loc", [P, 4], F32, isOutput=False)
    pri_in = nc.declare_dram_parameter("priors", [P, 4], F32, isOutput=False)
    val_out = nc.declare_dram_parameter("vals", [C, K], F32, isOutput=True)
    m_out = nc.declare_dram_parameter("m0", [C, NM], F32, isOutput=True)
    cv_out = nc.declare_dram_parameter("cval", [C, NC_], F32, isOutput=True)
    bv_out = nc.declare_dram_parameter("bval", [C, NB2], F32, isOutput=True)
    dec_out = nc.declare_dram_parameter("dec", [NCH, WIN * 4], F32, isOutput=True)

    from contextlib import ExitStack

    with TileContext(nc) as tc, ExitStack() as ctx:
        consts = ctx.enter_context(tc.tile_pool(name="consts", bufs=1))
        sb = ctx.enter_context(tc.tile_pool(name="sb", bufs=1))
        psum = ctx.enter_context(tc.tile_pool(name="psum", bufs=2, space="PSUM"))
        small = ctx.enter_context(tc.tile_pool(name="small", bufs=2))

        ident = consts.tile([NCH, NCH], F32)
        make_identity(nc, ident)

        # ---------------- conf load: two half-window tiles -----------------
        # h0 = priors [200p, 200p+100) per window, h1 = [200p+100, 200p+200).
        # Window 127 starts at P-WIN=25375 (overlapping window 126 by 25).
        # The two HWDGE queues (sync / scalar) carry ONLY conf: a dma_start
        # is a blocking DMA_DIRECT2D on the issuing engine, so any small-
        # descriptor DMA queued first would stall the conf load.
        conf_h = []
        full_view = conf_in[: FULLP * WIN, :].rearrange(
            "(p i) c -> p i c", p=FULLP)
        for h in range(2):
            t = sb.tile([NCH, HALF * C], F32, name=f"conf_h{h}")
            conf_h.append(t)
        # kill window-127's duplicated priors [25375, 25400) = first DUPN
        # prior-slices of conf_h0 partition 127. Compute-engine SBUF access
        # needs a 32-aligned partition base, so NEG-fill partitions [96:128)
        # first; the range DMAs below rewrite 96..126 and the h0 tail DMA
        # loads only the real priors [25400, 25475) into cols [DUPN*C:).
        nc.vector.memset(conf_h[0][96:NCH, : DUPN * C], NEG)
        # conf h0 on the sync queue, h1 on the scalar queue: the two HWDGE
        # queues feed disjoint DMA-engine groups, so splitting roughly
        # doubles load bandwidth and h1 lands while L1 h0 still computes.
        # <=64-partition slices keep 32.4KB-per-partition descriptor
        # coalescing (127-partition DMAs shatter into 1.6KB descriptors).
        # h0 split across BOTH queues so it gets all 16 DMA engines first;
        # a DVE gate copy (reads h0, writes a corner of h1's tile, which the
        # h1 DMAs then WAW-wait on) keeps h1 from stealing engine slots
        # until h0 has landed. The gate costs nothing on the DVE: it waits
        # on exactly the same h0 semaphores L1 h0 waits on.
        src_h0 = full_view[:, :HALF, :]
        src_h1 = full_view[:, HALF:, :]
        nc.sync.dma_start(out=conf_h[0][:64, :],
                          in_=src_h0[:64].rearrange("p i c -> p (i c)"))
        nc.scalar.dma_start(out=conf_h[0][64:FULLP, :],
                            in_=src_h0[64:].rearrange("p i c -> p (i c)"))
        nc.scalar.dma_start(
            out=conf_h[0][FULLP:NCH, DUPN * C :],
            in_=conf_in[P - WIN + DUPN : P - WIN + HALF, :]
            .rearrange("(p i) c -> p (i c)", p=1))
        nc.vector.tensor_copy(conf_h[1][:, 0:8], conf_h[0][:, 0:8])
        nc.sync.dma_start(out=conf_h[1][:64, :],
                          in_=src_h1[:64].rearrange("p i c -> p (i c)"))
        nc.scalar.dma_start(out=conf_h[1][64:FULLP, :],
                            in_=src_h1[64:].rearrange("p i c -> p (i c)"))
        nc.scalar.dma_start(
            out=conf_h[1][FULLP:NCH, :],
            in_=conf_in[P - WIN + HALF : P, :]
            .rearrange("(p i) c -> p (i c)", p=1))

        # ---------------- load loc / priors (both queues, after conf) ------
        # descriptor-bound (127 x 3.2KB): queued behind conf so the engine-
        # blocking DMA instructions never delay the conf stream.
        loc_sb = sb.tile([NCH, WIN * 4], F32)
        pri_sb = sb.tile([NCH, WIN * 4], F32)
        # gates: loc/pri descriptors would steal ~4 of 16 DMA engines from
        # the conf h1 stream; these GpSimd copies (overwritten by the loads)
        # make the loc/pri DMAs WAW-wait until h1 has landed.
        nc.gpsimd.tensor_copy(loc_sb[:, 0:64], conf_h[1][:, 0:64])
        nc.gpsimd.tensor_copy(pri_sb[:, 0:64], conf_h[1][:, 0:64])
        # partition 127 reads the OVERLAPPED full window [P-WIN, P); its
        # duplicated priors are neutralized by the conf_h0 memset above.
        for dst, src in ((loc_sb, loc_in), (pri_sb, pri_in)):
            nc.sync.dma_start(
                out=dst[:64, :],
                in_=src[: 64 * WIN, :].rearrange("(p i) c -> p (i c)", p=64),
            )
            nc.scalar.dma_start(
                out=dst[64:FULLP, :],
                in_=src[64 * WIN : FULLP * WIN, :]
                .rearrange("(p i) c -> p (i c)", p=FULLP - 64),
            )
            nc.scalar.dma_start(
                out=dst[FULLP:NCH, :],
                in_=src[P - WIN :, :].rearrange("(p i) c -> p (i c)", p=1),
            )

        # ---------------- L1 h0: per-class top-8 values ---------------------
        # cand_val[p, c*16 + 8h + r] = r-th largest of conf[half h of window p,
        # class c]. No index recovery on device (host does value search).
        cand_val = sb.tile([NCH, C * SLOT], F32)

        def l1_half(h):
            view = conf_h[h][:].rearrange("p (i c) -> p c i", c=C)
            for c in range(C):
                vdst = cand_val[:, c * SLOT + 8 * h : c * SLOT + 8 * h + 8]
                nc.vector.max(vdst, view[:, c, :])

        l1_half(0)
        l1_half(1)

        # ---------------- decode (GpSimd, off the DVE critical path) --------
        def coord(t, k):
            return t[:].rearrange("p (i c) -> p c i", c=4)[:, k, :]

        dec_sb = sb.tile([NCH, WIN * 4], F32)
        cxy = sb.tile([NCH, 2 * WIN], F32)
        wh = sb.tile([NCH, 2 * WIN], F32)
        tmps = [(sb.tile([NCH, WIN], F32, name=f"dtmp1_{k}"),
                 sb.tile([NCH, WIN], F32, name=f"dtmp2_{k}")) for k in range(2)]
        for k in range(2):  # k=0: x, k=1: y
            tmp1, tmp2 = tmps[k]
            Lp, Lwh = coord(loc_sb, k), coord(loc_sb, 2 + k)
            Pp, Pwh = coord(pri_sb, k), coord(pri_sb, 2 + k)
            cx = cxy[:, k * WIN : (k + 1) * WIN]
            w = wh[:, k * WIN : (k + 1) * WIN]
            # w = pw * exp(0.2 * lw)
            nc.gpsimd.tensor_copy(tmp1, Lwh)
            nc.scalar.activation(tmp1, tmp1, mybir.ActivationFunctionType.Exp,
                                 scale=VAR1)
            nc.gpsimd.tensor_mul(w, Pwh, tmp1)
            # cx = px + 0.1 * lx * pw
            nc.gpsimd.tensor_mul(tmp2, Lp, Pwh)
            nc.gpsimd.tensor_scalar_mul(tmp2, tmp2, VAR0)
            nc.gpsimd.tensor_add(cx, Pp, tmp2)
            # x1 = cx - w/2 ; x2 = x1 + w
            nc.gpsimd.tensor_scalar_mul(tmp2, w, 0.5)
            nc.gpsimd.tensor_sub(coord(dec_sb, k), cx, tmp2)
            nc.gpsimd.tensor_add(coord(dec_sb, 2 + k), coord(dec_sb, k), w)
        # dec stored window-flat [128, 800] (one contiguous 3.2KB descriptor
        # per partition); host reshapes. On sync: free after conf.
        nc.sync.dma_start(out=dec_out[:], in_=dec_sb[:])

        # ---------------- transpose candidates to class-major --------------
        val_T = sb.tile([C, NCH * SLOT], F32)
        sview = cand_val[:].rearrange("p (c s) -> p s c", s=SLOT)
        dview = val_T[:].rearrange("q (t s) -> q s t", s=SLOT)
        for grp in (1, 3, 0, 2):
            pt = psum.tile([C, 4 * NCH], F32, tag="tp")
            for k in range(4):
                s = grp * 4 + k
                nc.tensor.transpose(
                    pt[:, k * NCH : (k + 1) * NCH], sview[:, s, :], ident[:]
                )
            nc.scalar.copy(
                dview[:, grp * 4 : grp * 4 + 4, :],
                pt[:].rearrange("q (k t) -> q k t", k=4),
            )

        # t-major slot views: A: s in {0,1,8,9}, B: {2,3,10,11}, C: {4..7,12..15}
        def pool_view(t, s0):
            return t[:].rearrange("q (t h s) -> q t h s", h=2, s=8)[
                :, :, :, s0 : s0 + 2
            ]

        def poolC_view(t):
            return t[:].rearrange("q (t h s) -> q t h s", h=2, s=8)[:, :, :, 4:8]

        # ---------------- C-pool premerge: top-8 of 1024 --------------------
        # positions recovered host-side by stable argsort of the dumped pool
        # (same emulation as the master table), so no find_index8 anywhere.
        Cval = sb.tile([C, NC_], F32)
        nc.scalar.copy(Cval[:].rearrange("q (t h s) -> q t h s", h=2, s=4),
                       poolC_view(val_T))
        nc.sync.dma_start(out=cv_out[:], in_=Cval[:])
        c8val = small.tile([C, 8], F32, tag="c8v")
        nc.vector.max(c8val, Cval)

        # ---------------- B' = B + C8 premerge: top-24 ----------------------
        Bval = sb.tile([C, NB2], F32)
        nc.scalar.copy(Bval[:, :NB].rearrange("q (t h s) -> q t h s", h=2, s=2),
                       pool_view(val_T, 2))
        nc.vector.tensor_copy(Bval[:, NB:NB2], c8val)
        Bdump = sb.tile([C, NB2], F32)
        nc.scalar.copy(Bdump[:], Bval[:])
        nc.sync.dma_start(out=bv_out[:], in_=Bdump[:])

        b24val = sb.tile([C, NB24], F32)
        for r in range(3):
            vs = b24val[:, 8 * r : 8 * r + 8]
            nc.vector.max(vs, Bval)
            if r < 2:
                nc.vector.match_replace(Bval, vs, Bval, NEG)

        # ---------------- master = A + B24 ----------------------------------
        Mval = sb.tile([C, NM], F32)
        nc.scalar.copy(Mval[:, :NA].rearrange("q (t h s) -> q t h s", h=2, s=2),
                       pool_view(val_T, 0))
        nc.vector.tensor_copy(Mval[:, NA:NM], b24val)

        # dump the assembled master table (positions recovered host-side by
        # stable argsort -- exactly the max8/match-replace extraction order).
        # Copy first so round-1's match_replace doesn't wait on the DMA.
        Mdump = sb.tile([C, NM], F32)
        nc.scalar.copy(Mdump[:], Mval[:])
        nc.sync.dma_start(out=m_out[:], in_=Mdump[:])

        # ---------------- 25 extraction rounds (values only) ----------------
        # max8 writes straight into the vals slice; match_replace reads its
        # needles from the same slice -- no staging, no cross-engine traffic.
        vals_sb = sb.tile([C, K], F32)
        for r in range(ROUNDS):
            vs = vals_sb[:, 8 * r : 8 * r + 8]
            nc.vector.max(vs, Mval)
            nc.vector.match_replace(Mval, vs, Mval, NEG)
            if r == ROUNDS - 7:
                # rounds 0..18 done: ship the first 152 columns early,
                # partition-split across both queues
                nc.sync.dma_start(out=val_out[: C // 2, : 8 * (ROUNDS - 6)],
                                  in_=vals_sb[: C // 2, : 8 * (ROUNDS - 6)])
                nc.scalar.dma_start(
                    out=val_out[C // 2 :, : 8 * (ROUNDS - 6)],
                    in_=vals_sb[C // 2 :, : 8 * (ROUNDS - 6)])
            if r == ROUNDS - 3:
                # rounds 19..22: next 32 columns while the last rounds run
                nc.sync.dma_start(
                    out=val_out[: C // 2, 8 * (ROUNDS - 6) : 8 * (ROUNDS - 2)],
                    in_=vals_sb[: C // 2, 8 * (ROUNDS - 6) : 8 * (ROUNDS - 2)])
                nc.scalar.dma_start(
                    out=val_out[C // 2 :, 8 * (ROUNDS - 6) : 8 * (ROUNDS - 2)],
                    in_=vals_sb[C // 2 :, 8 * (ROUNDS - 6) : 8 * (ROUNDS - 2)])

        # last 16 columns: split across both queues to halve descriptor tail
        nc.sync.dma_start(out=val_out[: C // 2, 8 * (ROUNDS - 2) :],
                          in_=vals_sb[: C // 2, 8 * (ROUNDS - 2) :])
        nc.scalar.dma_start(out=val_out[C // 2 :, 8 * (ROUNDS - 2) :],
                            in_=vals_sb[C // 2 :, 8 * (ROUNDS - 2) :])

    if compile:
        nc.compile()
    return nc


_NC = None


def _get_nc():
    global _NC
    if _NC is None:
        _NC = build_nc()
    return _NC


def _install_ntff_shim():
    """The container's antenv lacks axon_hooks; synthesize it from the boot
    module's ctypes NTFF driver so trace=True can profile."""
    import types

    if "antenv.axon_hooks" in sys.modules:
        return
    try:
        from trn_agent_boot.trn_boot import _ntff_profile_via_ctypes

        hook = _ntff_profile_via_ctypes("/opt/axon/libaxon_pjrt.so")
    except Exception:
        hook = None
    mod = types.ModuleType("antenv.axon_hooks")
    mod._hook = hook
    mod.get_axon_ntff_profile_hook = lambda: mod._hook
    mod.set_axon_ntff_profile_hook = lambda h: setattr(mod, "_hook", h)
    sys.modules["antenv.axon_hooks"] = mod


def _decode_master_pos(qbuf, c8pos, b24pos):
    """Map master positions [C, K] -> (window t, half h) per winner.

    Master layout: pos < 512 -> A-pool (t*4 + h*2 + rank); pos >= 512 ->
    b24pos[pos-512] -> B' pool: < 512 -> B (t*4 + h*2 + rank), >= 512 ->
    c8pos[.-512] -> C-pool (t*8 + h*4 + rank).
    """
    m = qbuf.astype(np.int64)                      # [C, K]
    t = np.empty_like(m)
    hh = np.empty_like(m)

    inA = m < NA
    t[inA] = m[inA] // 4
    hh[inA] = (m[inA] // 2) % 2

    j = np.clip(m - NA, 0, NB24 - 1)
    p = np.take_along_axis(b24pos.astype(np.int64), j, axis=1)  # [C, K]
    inB = (~inA) & (p < NB)
    t[inB] = p[inB] // 4
    hh[inB] = (p[inB] // 2) % 2

    q = np.take_along_axis(c8pos.astype(np.int64), np.clip(p - NB, 0, 7), axis=1)
    inC = (~inA) & (p >= NB)
    t[inC] = q[inC] // 8
    hh[inC] = (q[inC] // 4) % 2
    return t, hh


def _resolve_prior_indices(conf_b, vals, t, hh):
    """Resolve each winner's prior index by exact value search in its
    100-prior half-window (emulating max8 + stable-tie semantics)."""
    Cn, Kn = vals.shape
    tf = t.ravel()
    hf = hh.ravel()
    cf = np.repeat(np.arange(Cn), Kn)
    vf = vals.ravel()

    # search domain [lo, hi): window start 200t (window 127 starts at 25375),
    # but window 127 half 0's first DUPN priors were killed on device.
    ws = np.where(tf < FULLP, WIN * tf, P - WIN)
    lo = ws + HALF * hf
    hi = lo + HALF
    t127h0 = (tf == FULLP) & (hf == 0)
    lo = np.where(t127h0, P - WIN + DUPN, lo)

    idx = lo[:, None] + np.arange(HALF)[None, :]          # [N, 100]
    idx_c = np.minimum(idx, P - 1)
    S = conf_b[idx_c, cf[:, None]]                        # gathered slices
    eq = (S == vf[:, None]) & (idx < hi[:, None])
    am = eq.argmax(axis=1)
    gidx = lo + am

    # duplicate winners in the same (c, lo) with the same value: assign
    # successive occurrences in output-rank order (stable)
    key = np.stack([cf, lo, vf.view(np.int32).astype(np.int64)], axis=1)
    _, inv, counts = np.unique(key, axis=0, return_inverse=True,
                               return_counts=True)
    dup_groups = np.flatnonzero(counts > 1)
    if dup_groups.size:
        for g in dup_groups:
            rows = np.flatnonzero(inv == g)               # in rank order
            occ = np.flatnonzero(eq[rows[0]])
            n = min(len(rows), len(occ))
            gidx[rows[:n]] = lo[rows[0]] + occ[:n]
    return gidx.reshape(Cn, Kn)


def _run(loc_data, conf_data, prior_data, trace=False):
    from concourse.bass_utils import run_bass_kernel_spmd

    if trace:
        _install_ntff_shim()

    nc = _get_nc()
    B = conf_data.shape[0]
    in_maps = [
        {
            "conf": np.ascontiguousarray(conf_data[b], dtype=np.float32),
            "loc": np.ascontiguousarray(loc_data[b], dtype=np.float32),
            "priors": np.ascontiguousarray(prior_data[0], dtype=np.float32),
        }
        for b in range(B)
    ]
    res = run_bass_kernel_spmd(nc, in_maps, list(range(B)), trace=trace)
    out = np.empty((B, C, K, 5), np.float32)
    for b in range(B):
        r = res.results[b]
        vals = np.asarray(r["vals"])              # [C, K] sorted desc
        m0 = np.asarray(r["m0"])                  # [C, NM] master table
        cval = np.asarray(r["cval"])              # [C, 1024] C pool
        bval = np.asarray(r["bval"])              # [C, 520]  B' pool
        # device extraction == stable descending order of each table
        qbuf = np.argsort(-m0, axis=1, kind="stable")[:, :K].astype(np.uint32)
        c8pos = np.argsort(-cval, axis=1, kind="stable")[:, :8]
        b24pos = np.argsort(-bval, axis=1, kind="stable")[:, :NB24]
        dec_flat = np.asarray(r["dec"]).reshape(NCH, WIN, 4)
        dec = np.concatenate(
            [dec_flat[:FULLP].reshape(-1, 4), dec_flat[FULLP, DUPN:]])
        t, hh = _decode_master_pos(qbuf, c8pos, b24pos)
        gidx = _resolve_prior_indices(in_maps[b]["conf"], vals, t, hh)
        # stable-order repair: adjacent equal values whose prior order is
        # inverted (cross-pool ties) are swapped to match jax.lax.top_k
        eq = vals[:, :-1] == vals[:, 1:]
        gt = gidx[:, :-1] > gidx[:, 1:]
        sw = np.where(eq & gt)
        l, rr = sw[0], sw[1]
        g2 = gidx.copy()
        g2[l, rr], g2[l, rr + 1] = gidx[l, rr + 1], gidx[l, rr]
        out[b, :, :, 0] = vals
        out[b, :, :, 1:] = dec[g2]
    return out, res


def kernel(loc_data, conf_data, prior_data):
    out, _ = _run(np.asarray(loc_data), np.asarray(conf_data),
                  np.asarray(prior_data))
    return out


# revision 36
# speedup vs baseline: 1.0304x; 1.0022x over previous
"""SSD Detect (decode + per-class top-200) Trainium2 Bass kernel.

Sharding: data-parallel over batch. 8 batches -> 8 NeuronCores, one batch per
core. Each core computes, for its batch:
  decoded boxes [25575, 4]  (SSD decode from loc + priors)
  per class c in [0, 81): top-200 scores (desc, ties -> lower prior index
  first, matching jax.lax.top_k) with their decoded boxes ->
  out[c, r] = [score_r, x1, y1, x2, y2]

Device algorithm per core:
  - conf is loaded as TWO half-window tiles (priors [200p, 200p+100) and
    [200p+100, 200p+200) per window), each half split across BOTH HWDGE
    queues so it uses all 16 DMA engines (~190GB/s). A DVE gate copy makes
    the h1 DMAs WAW-wait for h0, so h0 lands at ~30us with full bandwidth
    and L1 h0 overlaps the h1 load. Descriptor-heavy transfers (loc/priors:
    127 x 3.2KB; dma_start is a blocking DMA_DIRECT2D on its issuing
    engine) are queued after conf. <=64-partition slices keep 32.4KB-per-
    partition descriptor coalescing.
  - L1: per (class, half) DVE max8 gives the top-8 VALUES of each 100-prior
    half per window -- 162 max8 ops, NO find_index8/index tracking. Winner
    prior indices are recovered host-side by exact f32 value search inside
    the statically-known 100-prior half (emulating max8 + stable-tie
    semantics). Window 127 overlaps 126 by 25 priors; the duplicate region
    is NEG-killed via a pre-load memset (32-aligned partition base).
  - box decode runs on the otherwise-idle GpSimd engine (exp on Scalar),
    keeping the DVE stream pure L1+merge.
  - candidates PE-transposed to class-major [81, 2048] (t-major order).
  - 3-tier merge per class, values only:
      C-pool (half-ranks 4..7, 1024 slots) -> top-8
      B-pool (half-ranks 2..3, 512) + C8   -> top-24 (joint B+C winners
                                                      <= 24, verified)
      master = A-pool (half-ranks 0..1, 512) + B24 = 536
    25 rounds of (max8 -> vals slice, match_replace) extract the sorted
    top-200 values. No find_index8 anywhere: the premerge pools (Cval,
    B'val) and the assembled master table are dumped to DRAM; the host
    recovers every position with a stable descending argsort -- a
    bit-exact emulation of the max8/match-replace extraction order.
  - host: master pos -> (window, half) statically, prior index by value
    search, stable-order tie fix-up, box gather from the dumped decode.
"""

import sys

sys.path.insert(0, "/opt/trn_rl_repo")

import numpy as np

import concourse.bass as bass
import concourse.bacc as bacc
import concourse.mybir as mybir
from concourse.bass_types import AP  # noqa: F401
from concourse.masks import make_identity
from concourse.tile import TileContext

F32 = mybir.dt.float32
I32 = mybir.dt.int32
U32 = mybir.dt.uint32

P = 25575            # priors
C = 81               # classes
K = 200              # top-k
NCH = 128            # partitions / prior windows
WIN = 200            # priors per window
HALF = 100           # priors per half-window
NEG = -1.0e30
VAR0, VAR1 = 0.1, 0.2

SLOT = 16            # candidate slots per class per partition
NA, NB, NC_ = 512, 512, 1024   # pool sizes per class
NB2 = NB + 8         # B' = B + C8
NB24 = 24            # B' premerge keep (joint B+C winners <= 24, verified)
NM = NA + NB24       # master size
ROUNDS = 25

FULLP = NCH - 1      # partitions with full windows (127)
TAILI = P - FULLP * WIN   # real priors in the last window (175)
DUPN = WIN - TAILI        # duplicated priors at start of window 127 (25)


def build_nc(compile=True):
    nc = bacc.Bacc()
    conf_in = nc.declare_dram_parameter("conf", [P, C], F32, isOutput=False)
    loc_in = nc.declare_dram_parameter("loc", [P, 4], F32, isOutput=False)
    pri_in = nc.declare_dram_parameter("priors", [P, 4], F32, isOutput=False)
    val_out = nc.declare_dram_parameter("vals", [C, K], F32, isOutput=True)
    m_out = nc.declare_dram_parameter("m0", [C, NM], F32, isOutput=True)
    cv_out = nc.declare_dram_parameter("cval", [C, NC_], F32, isOutput=True)
    bv_out = nc.declare_dram_parameter("bval", [C, NB2], F32, isOutput=True)
    dec_out = nc.declare_dram_parameter("dec", [NCH, WIN * 4], F32, isOutput=True)

    from contextlib import ExitStack

    with TileContext(nc) as tc, ExitStack() as ctx:
        consts = ctx.enter_context(tc.tile_pool(name="consts", bufs=1))
        sb = ctx.enter_context(tc.tile_pool(name="sb", bufs=1))
        psum = ctx.enter_context(tc.tile_pool(name="psum", bufs=1, space="PSUM"))
        small = ctx.enter_context(tc.tile_pool(name="small", bufs=2))

        ident = consts.tile([NCH, NCH], F32)
        make_identity(nc, ident)

        # ---------------- conf load: two half-window tiles -----------------
        # h0 = priors [200p, 200p+100) per window, h1 = [200p+100, 200p+200).
        # Window 127 starts at P-WIN=25375 (overlapping window 126 by 25).
        # The two HWDGE queues (sync / scalar) carry ONLY conf: a dma_start
        # is a blocking DMA_DIRECT2D on the issuing engine, so any small-
        # descriptor DMA queued first would stall the conf load.
        conf_h = []
        full_view = conf_in[: FULLP * WIN, :].rearrange(
            "(p i) c -> p i c", p=FULLP)
        for h in range(2):
            t = sb.tile([NCH, HALF * C], F32, name=f"conf_h{h}")
            conf_h.append(t)
        # kill window-127's duplicated priors [25375, 25400) = first DUPN
        # prior-slices of conf_h0 partition 127. Compute-engine SBUF access
        # needs a 32-aligned partition base, so NEG-fill partitions [96:128)
        # first; the range DMAs below rewrite 96..126 and the h0 tail DMA
        # loads only the real priors [25400, 25475) into cols [DUPN*C:).
        nc.vector.memset(conf_h[0][96:NCH, : DUPN * C], NEG)
        # conf h0 on the sync queue, h1 on the scalar queue: the two HWDGE
        # queues feed disjoint DMA-engine groups, so splitting roughly
        # doubles load bandwidth and h1 lands while L1 h0 still computes.
        # <=64-partition slices keep 32.4KB-per-partition descriptor
        # coalescing (127-partition DMAs shatter into 1.6KB descriptors).
        # h0 split across BOTH queues so it gets all 16 DMA engines first;
        # a DVE gate copy (reads h0, writes a corner of h1's tile, which the
        # h1 DMAs then WAW-wait on) keeps h1 from stealing engine slots
        # until h0 has landed. The gate costs nothing on the DVE: it waits
        # on exactly the same h0 semaphores L1 h0 waits on.
        src_h0 = full_view[:, :HALF, :]
        src_h1 = full_view[:, HALF:, :]
        nc.sync.dma_start(out=conf_h[0][:64, :],
                          in_=src_h0[:64].rearrange("p i c -> p (i c)"))
        nc.scalar.dma_start(out=conf_h[0][64:FULLP, :],
                            in_=src_h0[64:].rearrange("p i c -> p (i c)"))
        nc.scalar.dma_start(
            out=conf_h[0][FULLP:NCH, DUPN * C :],
            in_=conf_in[P - WIN + DUPN : P - WIN + HALF, :]
            .rearrange("(p i) c -> p (i c)", p=1))
        nc.vector.tensor_copy(conf_h[1][:, 0:8], conf_h[0][:, 0:8])
        nc.sync.dma_start(out=conf_h[1][:64, :],
                          in_=src_h1[:64].rearrange("p i c -> p (i c)"))
        nc.scalar.dma_start(out=conf_h[1][64:FULLP, :],
                            in_=src_h1[64:].rearrange("p i c -> p (i c)"))
        nc.scalar.dma_start(
            out=conf_h[1][FULLP:NCH, :],
            in_=conf_in[P - WIN + HALF : P, :]
            .rearrange("(p i) c -> p (i c)", p=1))

        # ---------------- load loc / priors (both queues, after conf) ------
        # descriptor-bound (127 x 3.2KB): queued behind conf so the engine-
        # blocking DMA instructions never delay the conf stream.
        loc_sb = sb.tile([NCH, WIN * 4], F32)
        pri_sb = sb.tile([NCH, WIN * 4], F32)
        # gates: loc/pri descriptors would steal ~4 of 16 DMA engines from
        # the conf h1 stream; these GpSimd copies (overwritten by the loads)
        # make the loc/pri DMAs WAW-wait until h1 has landed.
        nc.gpsimd.tensor_copy(loc_sb[:, 0:64], conf_h[1][:, 0:64])
        nc.gpsimd.tensor_copy(pri_sb[:, 0:64], conf_h[1][:, 0:64])
        # partition 127 reads the OVERLAPPED full window [P-WIN, P); its
        # duplicated priors are neutralized by the conf_h0 memset above.
        for dst, src in ((loc_sb, loc_in), (pri_sb, pri_in)):
            nc.sync.dma_start(
                out=dst[:64, :],
                in_=src[: 64 * WIN, :].rearrange("(p i) c -> p (i c)", p=64),
            )
            nc.scalar.dma_start(
                out=dst[64:FULLP, :],
                in_=src[64 * WIN : FULLP * WIN, :]
                .rearrange("(p i) c -> p (i c)", p=FULLP - 64),
            )
            nc.scalar.dma_start(
                out=dst[FULLP:NCH, :],
                in_=src[P - WIN :, :].rearrange("(p i) c -> p (i c)", p=1),
            )

        # ---------------- L1 h0: per-class top-8 values ---------------------
        # cand_val[p, c*16 + 8h + r] = r-th largest of conf[half h of window p,
        # class c]. No index recovery on device (host does value search).
        cand_val = sb.tile([NCH, C * SLOT], F32)

        def l1_half(h):
            view = conf_h[h][:].rearrange("p (i c) -> p c i", c=C)
            for c in range(C):
                vdst = cand_val[:, c * SLOT + 8 * h : c * SLOT + 8 * h + 8]
                nc.vector.max(vdst, view[:, c, :])

        l1_half(0)
        l1_half(1)

        # ---------------- decode (GpSimd, off the DVE critical path) --------
        def coord(t, k):
            return t[:].rearrange("p (i c) -> p c i", c=4)[:, k, :]

        dec_sb = sb.tile([NCH, WIN * 4], F32)
        cxy = sb.tile([NCH, 2 * WIN], F32)
        wh = sb.tile([NCH, 2 * WIN], F32)
        tmps = [(sb.tile([NCH, WIN], F32, name=f"dtmp1_{k}"),
                 sb.tile([NCH, WIN], F32, name=f"dtmp2_{k}")) for k in range(2)]
        for k in range(2):  # k=0: x, k=1: y
            tmp1, tmp2 = tmps[k]
            Lp, Lwh = coord(loc_sb, k), coord(loc_sb, 2 + k)
            Pp, Pwh = coord(pri_sb, k), coord(pri_sb, 2 + k)
            cx = cxy[:, k * WIN : (k + 1) * WIN]
            w = wh[:, k * WIN : (k + 1) * WIN]
            # w = pw * exp(0.2 * lw)
            nc.gpsimd.tensor_copy(tmp1, Lwh)
            nc.scalar.activation(tmp1, tmp1, mybir.ActivationFunctionType.Exp,
                                 scale=VAR1)
            nc.gpsimd.tensor_mul(w, Pwh, tmp1)
            # cx = px + 0.1 * lx * pw
            nc.gpsimd.tensor_mul(tmp2, Lp, Pwh)
            nc.gpsimd.tensor_scalar_mul(tmp2, tmp2, VAR0)
            nc.gpsimd.tensor_add(cx, Pp, tmp2)
            # x1 = cx - w/2 ; x2 = x1 + w
            nc.gpsimd.tensor_scalar_mul(tmp2, w, 0.5)
            nc.gpsimd.tensor_sub(coord(dec_sb, k), cx, tmp2)
            nc.gpsimd.tensor_add(coord(dec_sb, 2 + k), coord(dec_sb, k), w)
        # dec stored window-flat [128, 800] (one contiguous 3.2KB descriptor
        # per partition); host reshapes. On sync: free after conf.
        nc.sync.dma_start(out=dec_out[:], in_=dec_sb[:])

        # ---------------- transpose candidates to class-major --------------
        # all four slot-group transposes stay live in PSUM (bufs=4); the
        # pool-assembly copies read PSUM directly -- no val_T staging tile,
        # no PSUM-evacuation copies on the critical premerge chain.
        # slot s = 8h + rank: grp = s//4, k = s%4. A: ranks {0,1}, B: {2,3},
        # C: {4..7}; h0 slots in grps 0/1, h1 slots in grps 2/3.
        sview = cand_val[:].rearrange("p (c s) -> p s c", s=SLOT)
        pts = {}
        for grp in (1, 3, 0, 2):
            pt = psum.tile([C, 4 * NCH], F32, tag=f"tp{grp}")
            for k in range(4):
                s = grp * 4 + k
                nc.tensor.transpose(
                    pt[:, k * NCH : (k + 1) * NCH], sview[:, s, :], ident[:]
                )
            pts[grp] = pt

        def grp_view(grp):
            return pts[grp][:].rearrange("q (k t) -> q t k", k=4)

        # ---------------- C-pool premerge: top-8 of 1024 --------------------
        # positions recovered host-side by stable argsort of the dumped pool
        # (same emulation as the master table), so no find_index8 anywhere.
        Cval = sb.tile([C, NC_], F32)
        for hh, g in ((0, 1), (1, 3)):
            nc.scalar.copy(
                Cval[:].rearrange("q (t h s) -> q t h s", h=2, s=4)[:, :, hh, :],
                grp_view(g))
        nc.sync.dma_start(out=cv_out[:], in_=Cval[:])
        c8val = small.tile([C, 8], F32, tag="c8v")
        nc.vector.max(c8val, Cval)

        # ---------------- B' = B + C8 premerge: top-24 ----------------------
        Bval = sb.tile([C, NB2], F32)
        for hh, g in ((0, 0), (1, 2)):
            nc.scalar.copy(
                Bval[:, :NB].rearrange("q (t h s) -> q t h s", h=2, s=2)
                [:, :, hh, :],
                grp_view(g)[:, :, 2:4])
        nc.vector.tensor_copy(Bval[:, NB:NB2], c8val)
        Bdump = sb.tile([C, NB2], F32)
        nc.scalar.copy(Bdump[:], Bval[:])
        nc.sync.dma_start(out=bv_out[:], in_=Bdump[:])

        b24val = sb.tile([C, NB24], F32)
        for r in range(3):
            vs = b24val[:, 8 * r : 8 * r + 8]
            nc.vector.max(vs, Bval)
            if r < 2:
                nc.vector.match_replace(Bval, vs, Bval, NEG)

        # ---------------- master = A + B24 ----------------------------------
        Mval = sb.tile([C, NM], F32)
        for hh, g in ((0, 0), (1, 2)):
            nc.scalar.copy(
                Mval[:, :NA].rearrange("q (t h s) -> q t h s", h=2, s=2)
                [:, :, hh, :],
                grp_view(g)[:, :, 0:2])
        nc.vector.tensor_copy(Mval[:, NA:NM], b24val)

        # dump the assembled master table (positions recovered host-side by
        # stable argsort -- exactly the max8/match-replace extraction order).
        # Copy first so round-1's match_replace doesn't wait on the DMA.
        Mdump = sb.tile([C, NM], F32)
        nc.scalar.copy(Mdump[:], Mval[:])
        nc.sync.dma_start(out=m_out[:], in_=Mdump[:])

        # ---------------- 25 extraction rounds (values only) ----------------
        # max8 writes straight into the vals slice; match_replace reads its
        # needles from the same slice -- no staging, no cross-engine traffic.
        vals_sb = sb.tile([C, K], F32)
        for r in range(ROUNDS):
            vs = vals_sb[:, 8 * r : 8 * r + 8]
            nc.vector.max(vs, Mval)
            nc.vector.match_replace(Mval, vs, Mval, NEG)
            if r == ROUNDS - 7:
                # rounds 0..18 done: ship the first 152 columns early,
                # partition-split across both queues
                nc.sync.dma_start(out=val_out[: C // 2, : 8 * (ROUNDS - 6)],
                                  in_=vals_sb[: C // 2, : 8 * (ROUNDS - 6)])
                nc.scalar.dma_start(
                    out=val_out[C // 2 :, : 8 * (ROUNDS - 6)],
                    in_=vals_sb[C // 2 :, : 8 * (ROUNDS - 6)])
            if r == ROUNDS - 3:
                # rounds 19..22: next 32 columns while the last rounds run
                nc.sync.dma_start(
                    out=val_out[: C // 2, 8 * (ROUNDS - 6) : 8 * (ROUNDS - 2)],
                    in_=vals_sb[: C // 2, 8 * (ROUNDS - 6) : 8 * (ROUNDS - 2)])
                nc.scalar.dma_start(
                    out=val_out[C // 2 :, 8 * (ROUNDS - 6) : 8 * (ROUNDS - 2)],
                    in_=vals_sb[C // 2 :, 8 * (ROUNDS - 6) : 8 * (ROUNDS - 2)])

        # last 16 columns: split across both queues to halve descriptor tail
        nc.sync.dma_start(out=val_out[: C // 2, 8 * (ROUNDS - 2) :],
                          in_=vals_sb[: C // 2, 8 * (ROUNDS - 2) :])
        nc.scalar.dma_start(out=val_out[C // 2 :, 8 * (ROUNDS - 2) :],
                            in_=vals_sb[C // 2 :, 8 * (ROUNDS - 2) :])

    if compile:
        nc.compile()
    return nc


_NC = None


def _get_nc():
    global _NC
    if _NC is None:
        _NC = build_nc()
    return _NC


def _install_ntff_shim():
    """The container's antenv lacks axon_hooks; synthesize it from the boot
    module's ctypes NTFF driver so trace=True can profile."""
    import types

    if "antenv.axon_hooks" in sys.modules:
        return
    try:
        from trn_agent_boot.trn_boot import _ntff_profile_via_ctypes

        hook = _ntff_profile_via_ctypes("/opt/axon/libaxon_pjrt.so")
    except Exception:
        hook = None
    mod = types.ModuleType("antenv.axon_hooks")
    mod._hook = hook
    mod.get_axon_ntff_profile_hook = lambda: mod._hook
    mod.set_axon_ntff_profile_hook = lambda h: setattr(mod, "_hook", h)
    sys.modules["antenv.axon_hooks"] = mod


def _decode_master_pos(qbuf, c8pos, b24pos):
    """Map master positions [C, K] -> (window t, half h) per winner.

    Master layout: pos < 512 -> A-pool (t*4 + h*2 + rank); pos >= 512 ->
    b24pos[pos-512] -> B' pool: < 512 -> B (t*4 + h*2 + rank), >= 512 ->
    c8pos[.-512] -> C-pool (t*8 + h*4 + rank).
    """
    m = qbuf.astype(np.int64)                      # [C, K]
    t = np.empty_like(m)
    hh = np.empty_like(m)

    inA = m < NA
    t[inA] = m[inA] // 4
    hh[inA] = (m[inA] // 2) % 2

    j = np.clip(m - NA, 0, NB24 - 1)
    p = np.take_along_axis(b24pos.astype(np.int64), j, axis=1)  # [C, K]
    inB = (~inA) & (p < NB)
    t[inB] = p[inB] // 4
    hh[inB] = (p[inB] // 2) % 2

    q = np.take_along_axis(c8pos.astype(np.int64), np.clip(p - NB, 0, 7), axis=1)
    inC = (~inA) & (p >= NB)
    t[inC] = q[inC] // 8
    hh[inC] = (q[inC] // 4) % 2
    return t, hh


def _resolve_prior_indices(conf_b, vals, t, hh):
    """Resolve each winner's prior index by exact value search in its
    100-prior half-window (emulating max8 + stable-tie semantics)."""
    Cn, Kn = vals.shape
    tf = t.ravel()
    hf = hh.ravel()
    cf = np.repeat(np.arange(Cn), Kn)
    vf = vals.ravel()

    # search domain [lo, hi): window start 200t (window 127 starts at 25375),
    # but window 127 half 0's first DUPN priors were killed on device.
    ws = np.where(tf < FULLP, WIN * tf, P - WIN)
    lo = ws + HALF * hf
    hi = lo + HALF
    t127h0 = (tf == FULLP) & (hf == 0)
    lo = np.where(t127h0, P - WIN + DUPN, lo)

    idx = lo[:, None] + np.arange(HALF)[None, :]          # [N, 100]
    idx_c = np.minimum(idx, P - 1)
    S = conf_b[idx_c, cf[:, None]]                        # gathered slices
    eq = (S == vf[:, None]) & (idx < hi[:, None])
    am = eq.argmax(axis=1)
    gidx = lo + am

    # duplicate winners in the same (c, lo) with the same value: assign
    # successive occurrences in output-rank order (stable)
    key = np.stack([cf, lo, vf.view(np.int32).astype(np.int64)], axis=1)
    _, inv, counts = np.unique(key, axis=0, return_inverse=True,
                               return_counts=True)
    dup_groups = np.flatnonzero(counts > 1)
    if dup_groups.size:
        for g in dup_groups:
            rows = np.flatnonzero(inv == g)               # in rank order
            occ = np.flatnonzero(eq[rows[0]])
            n = min(len(rows), len(occ))
            gidx[rows[:n]] = lo[rows[0]] + occ[:n]
    return gidx.reshape(Cn, Kn)


def _run(loc_data, conf_data, prior_data, trace=False):
    from concourse.bass_utils import run_bass_kernel_spmd

    if trace:
        _install_ntff_shim()

    nc = _get_nc()
    B = conf_data.shape[0]
    in_maps = [
        {
            "conf": np.ascontiguousarray(conf_data[b], dtype=np.float32),
            "loc": np.ascontiguousarray(loc_data[b], dtype=np.float32),
            "priors": np.ascontiguousarray(prior_data[0], dtype=np.float32),
        }
        for b in range(B)
    ]
    res = run_bass_kernel_spmd(nc, in_maps, list(range(B)), trace=trace)
    out = np.empty((B, C, K, 5), np.float32)
    for b in range(B):
        r = res.results[b]
        vals = np.asarray(r["vals"])              # [C, K] sorted desc
        m0 = np.asarray(r["m0"])                  # [C, NM] master table
        cval = np.asarray(r["cval"])              # [C, 1024] C pool
        bval = np.asarray(r["bval"])              # [C, 520]  B' pool
        # device extraction == stable descending order of each table
        qbuf = np.argsort(-m0, axis=1, kind="stable")[:, :K].astype(np.uint32)
        c8pos = np.argsort(-cval, axis=1, kind="stable")[:, :8]
        b24pos = np.argsort(-bval, axis=1, kind="stable")[:, :NB24]
        dec_flat = np.asarray(r["dec"]).reshape(NCH, WIN, 4)
        dec = np.concatenate(
            [dec_flat[:FULLP].reshape(-1, 4), dec_flat[FULLP, DUPN:]])
        t, hh = _decode_master_pos(qbuf, c8pos, b24pos)
        gidx = _resolve_prior_indices(in_maps[b]["conf"], vals, t, hh)
        # stable-order repair: adjacent equal values whose prior order is
        # inverted (cross-pool ties) are swapped to match jax.lax.top_k
        eq = vals[:, :-1] == vals[:, 1:]
        gt = gidx[:, :-1] > gidx[:, 1:]
        sw = np.where(eq & gt)
        l, rr = sw[0], sw[1]
        g2 = gidx.copy()
        g2[l, rr], g2[l, rr + 1] = gidx[l, rr + 1], gidx[l, rr]
        out[b, :, :, 0] = vals
        out[b, :, :, 1:] = dec[g2]
    return out, res


def kernel(loc_data, conf_data, prior_data):
    out, _ = _run(np.asarray(loc_data), np.asarray(conf_data),
                  np.asarray(prior_data))
    return out
